# revision 1
# baseline (speedup 1.0000x reference)
"""Trainium2 Bass kernel for nn_Block_3616362463321 (dense transformer block), v2.

B=8, T=1024, C=1024, H=16, Dh=64. Data-parallel: core b computes batch elem b.

Key design vs v1:
- bf16 everywhere on device (matmuls stream 1 row/cycle same as f32r, DVE gets
  2x modes, DMA bytes halve). Host casts weights/x to bf16 (free).
- x is shipped TRANSPOSED from host (xT [C,T]) -> zero PE transposes.
- LayerNorm gains/biases folded into weights host-side:
    xn1 = g1*(x-m)/std + be1; Q = xn1@Wq = z@(g1*Wq) + be1@Wq, z=(x-m)*r
  be-terms become ACT-copy bias columns (Q,K) / an extra K=1 matmul row (V).
- LN2 fully folded into the MLP:  h = relu(z2@W1''+b1') = r2 * relu(Ã),
    Ã = y@W1'' + (-c1)(x)m2 + b1'(x)std2   (2 extra contraction rows)
    out = y + r2*(relu(Ã)@W2 + b2(x)std2)  (1 extra row; r2 applied once)
- LN stats via ones-column matmuls on PE + ACT squares (no bn_stats passes).
- Causal mask applied as a 0/1 triangle multiply on the bf16 E tile (DVE 2x
  mode) after exp; fully-masked blocks are never computed.
- Softmax denominators via V's leading ones-column (sum lands at partition 0
  of each A^T tile): DVE reciprocal in place, PE ones-matmul broadcast,
  DVE multiply, then a host-shipped shifted-identity matmul realigns rows
  1:65 of both heads into attnT[0:64]/[64:128] (2-deep software pipeline so
  the PE never waits on the normalize chain).
- One DMA per weight tensor, host pre-shuffled to per-partition-contiguous
  [P, KC*C] layout: 128 DMA descriptors each instead of 1024 (descriptor
  generation was ~35us per gathered DMA), ~20 DMA instructions total vs ~200.
- No gpsimd compute anywhere: Pool partition_broadcast/tensor_tensor measure
  ~5-10us per op on HW (software dispatched).
- Output stored transposed (outT [C,T] bf16); host transposes/casts back.
"""
import sys

sys.path.insert(0, "/opt/trn_rl_repo")

from contextlib import ExitStack, nullcontext

import numpy as np
import ml_dtypes

import concourse.bacc as bacc
import concourse.bass as bass
import concourse.mybir as mybir
import concourse.tile as tile
from concourse.bass_utils import run_bass_kernel_spmd

P = 128
B, T, C, H = 8, 1024, 1024, 16
Dh = C // H            # 64
EPS = 1e-5
NF = 512               # PSUM bank free dim (fp32)
KC = C // P            # 8 chunks of 128
TJ = T // P            # 8 t-blocks of 128
TN = T // NF           # 2 t-halves of 512
F32 = mybir.dt.float32
BF16 = mybir.dt.bfloat16
ALU = mybir.AluOpType
ACTF = mybir.ActivationFunctionType

N_CORES = 8
_CACHE = {}


def build_nc(loop=1, hwloop=0, phases=7):
    nc = bacc.Bacc("TRN2", target_bir_lowering=False, debug=False)

    xt_d = nc.dram_tensor("xt", [P, KC * T], BF16, kind="ExternalInput")
    wq_d = nc.dram_tensor("wq", [P, KC * C], BF16, kind="ExternalInput")
    wk_d = nc.dram_tensor("wk", [P, KC * C], BF16, kind="ExternalInput")
    wv_d = nc.dram_tensor("wv", [P, KC * C], BF16, kind="ExternalInput")
    wp_d = nc.dram_tensor("wp", [P, KC * C], BF16, kind="ExternalInput")
    w1_d = nc.dram_tensor("w1", [P, KC * C], BF16, kind="ExternalInput")
    w2_d = nc.dram_tensor("w2", [P, KC * C], BF16, kind="ExternalInput")
    # bias columns, fp32: [:,0:8]=beq  [:,8:16]=bek  [:,16:24]=bproj
    cols_d = nc.dram_tensor("cols", [P, 24], F32, kind="ExternalInput")
    # extra contraction rows, bf16: 0:-c1  1:b1'  2:b2  3:bev
    ext_d = nc.dram_tensor("ext", [4, C], BF16, kind="ExternalInput")
    # [:,0:128]=ident [:,128:256]=causal 0/1 tri [:,256:320]=row-shift
    consts_d = nc.dram_tensor("consts", [P, 2 * P + 64], BF16,
                              kind="ExternalInput")
    out_d = nc.dram_tensor("out", [C, T], BF16, kind="ExternalOutput")

    with tile.TileContext(nc) as tc, ExitStack() as ES:
        singles = ES.enter_context(tc.tile_pool(name="singles", bufs=1))
        arena = ES.enter_context(tc.tile_pool(name="arena", bufs=1))
        psum = ES.enter_context(tc.tile_pool(name="psum", bufs=1, space="PSUM"))

        _si = [0]
        _ai = [0]

        def stile(nm="s", shape=(P, NF)):
            t = psum.tile(list(shape), F32, tag=f"S{_si[0] % 4}",
                          name=f"{nm}{_si[0]}")
            _si[0] += 1
            return t

        def atile(nm="a"):
            t = psum.tile([P, NF], F32, tag=f"A{_ai[0] % 4}", name=f"{nm}{_ai[0]}")
            _ai[0] += 1
            return t

        def big(tag, nm, shape=(P, KC, T), dtype=BF16):
            return arena.tile(list(shape), dtype, tag=tag, name=nm)

        _dq = [0]

        def bulk_dma(out, in_):
            eng = nc.sync if _dq[0] % 2 == 0 else nc.scalar
            _dq[0] += 1
            eng.dma_start(out=out, in_=in_)

        # ---- constants / small tiles ----
        consts = singles.tile([P, 2 * P + 64], BF16)
        nc.scalar.dma_start(out=consts[:], in_=consts_d[:, :])
        cols = singles.tile([P, 24], F32)
        nc.scalar.dma_start(out=cols[:], in_=cols_d[:, :])
        ext01 = singles.tile([2, C], BF16)   # rows: -c1, b1'
        nc.scalar.dma_start(out=ext01[:], in_=ext_d[0:2, :])
        extb2 = singles.tile([1, C], BF16)   # b2
        nc.scalar.dma_start(out=extb2[:], in_=ext_d[2:3, :])
        extbv = singles.tile([1, C], BF16)   # bev
        nc.scalar.dma_start(out=extbv[:], in_=ext_d[3:4, :])
        onesC = singles.tile([P, 1], BF16)
        nc.vector.memset(onesC[:], 1.0 / C)
        onesR = singles.tile([1, P], BF16)
        nc.vector.memset(onesR[:], 1.0)
        epsc = singles.tile([P, 1], F32)
        nc.vector.memset(epsc[:], EPS)

        # LN stat rows / broadcast tiles
        m1row = singles.tile([1, T], BF16)
        r1row = singles.tile([1, T], BF16)
        mrows = singles.tile([2, T], BF16)   # p0: m2, p1: std2
        std2row = singles.tile([1, T], BF16)
        r2row = singles.tile([1, T], BF16)
        r2b = singles.tile([P, T], BF16)

        def ident():
            return consts[:, 0:P]

        def maskS():
            return consts[:, P:2 * P]

        def shiftS():
            return consts[0:Dh + 1, 2 * P:2 * P + Dh]

        with (tc.For_i(0, hwloop, 1) if hwloop else nullcontext()):
            for _it in range(loop):
                # ---------------- load x^T + first weights ----------------
                xt = big("T5", "xt")
                nc.sync.dma_start(
                    out=xt[:], in_=xt_d.ap().rearrange("p (k t) -> p k t", k=KC))
                wq = big("T2", "wq")
                bulk_dma(out=wq[:],
                         in_=wq_d.ap().rearrange("p (k n) -> p k n", k=KC))
                wk = big("T3", "wk")
                bulk_dma(out=wk[:],
                         in_=wk_d.ap().rearrange("p (k n) -> p k n", k=KC))
                wv = big("T4", "wv")
                bulk_dma(out=wv[:],
                         in_=wv_d.ap().rearrange("p (k n) -> p k n", k=KC))

                # ---------------- LN1 stats ----------------
                LN1 = ExitStack()
                ln1bc = LN1.enter_context(tc.tile_pool(name="ln1bc", bufs=1))
                m1b = ln1bc.tile([P, T], BF16, tag="m1b", name="m1b")
                r1b = ln1bc.tile([P, T], BF16, tag="r1b", name="r1b")
                sq = big("T0", "sq")
                for k in range(KC):
                    nc.scalar.activation(out=sq[:, k, :], in_=xt[:, k, :],
                                         func=ACTF.Square, scale=1.0)
                with ExitStack() as S:
                    rtmp = S.enter_context(tc.tile_pool(name="rtmp", bufs=2))
                    for tn in range(TN):
                        tsl = slice(tn * NF, (tn + 1) * NF)
                        ps_m = stile("psm", (1, NF))
                        for k in range(KC):
                            nc.tensor.matmul(ps_m[:], lhsT=onesC[:],
                                             rhs=xt[:, k, tsl],
                                             start=(k == 0), stop=(k == KC - 1))
                        ps_s2 = stile("pss", (1, NF))
                        for k in range(KC):
                            nc.tensor.matmul(ps_s2[:], lhsT=onesC[:],
                                             rhs=sq[:, k, tsl],
                                             start=(k == 0), stop=(k == KC - 1))
                        msq = rtmp.tile([1, NF], F32, tag="msq")
                        nc.scalar.activation(out=msq[:], in_=ps_m[:],
                                             func=ACTF.Square, scale=1.0)
                        var = rtmp.tile([1, NF], F32, tag="var")
                        nc.vector.scalar_tensor_tensor(
                            out=var[:], in0=ps_s2[:], scalar=1.0, in1=msq[:],
                            op0=ALU.mult, op1=ALU.subtract)
                        stdr = rtmp.tile([1, NF], F32, tag="stdr")
                        nc.scalar.activation(out=stdr[:], in_=var[:],
                                             func=ACTF.Sqrt,
                                             bias=epsc[0:1, :], scale=1.0)
                        with nc.allow_low_precision(reason="ln rstd bf16"):
                            nc.vector.reciprocal(r1row[0:1, tsl], stdr[:])
                        nc.scalar.activation(out=m1row[0:1, tsl], in_=ps_m[:],
                                             func=ACTF.Copy, scale=1.0)
                        bc0 = stile("bc0")
                        nc.tensor.matmul(bc0[:], lhsT=onesR[0:1, :],
                                         rhs=m1row[0:1, tsl],
                                         start=True, stop=True)
                        nc.scalar.activation(out=m1b[:, tsl], in_=bc0[:],
                                             func=ACTF.Copy, scale=1.0)
                        bc1 = stile("bc1")
                        nc.tensor.matmul(bc1[:], lhsT=onesR[0:1, :],
                                         rhs=r1row[0:1, tsl],
                                         start=True, stop=True)
                        nc.scalar.activation(out=r1b[:, tsl], in_=bc1[:],
                                             func=ACTF.Copy, scale=1.0)

                # ---------------- z = (x^T - m1)*r1 ----------------
                z = big("T1", "z")
                for k in range(KC):
                    nc.vector.tensor_tensor(z[:, k, :], xt[:, k, :], m1b[:, :],
                                            ALU.subtract)
                    nc.vector.tensor_tensor(z[:, k, :], z[:, k, :], r1b[:, :],
                                            ALU.mult)
                LN1.close()

                # ---------------- QKV ----------------
                if phases < 2:
                    for m in range(KC):
                        bulk_dma(out=out_d[m * P:(m + 1) * P, :],
                                 in_=z[:, m, :])
                    continue
                QT = big("T6", "QT")
                KT = big("T7", "KT")
                for tn in range(TN):
                    tsl = slice(tn * NF, (tn + 1) * NF)
                    for m in range(KC):
                        msl = slice(m * P, (m + 1) * P)
                        pq = stile("pq")
                        for k in range(KC):
                            nc.tensor.matmul(pq[:], lhsT=wq[:, k, msl],
                                             rhs=z[:, k, tsl],
                                             start=(k == 0), stop=(k == KC - 1))
                        nc.scalar.activation(out=QT[:, m, tsl], in_=pq[:],
                                             func=ACTF.Identity,
                                             bias=cols[:, m:m + 1], scale=1.0)
                        pk = stile("pk")
                        for k in range(KC):
                            nc.tensor.matmul(pk[:], lhsT=wk[:, k, msl],
                                             rhs=z[:, k, tsl],
                                             start=(k == 0), stop=(k == KC - 1))
                        nc.scalar.activation(out=KT[:, m, tsl], in_=pk[:],
                                             func=ACTF.Identity,
                                             bias=cols[:, 8 + m:9 + m], scale=1.0)

                # V natural [t(part), j, h, 65]; even heads [d|1], odd [1|d]
                V = big("TV", "V", (P, TJ, H, Dh + 1))
                nc.vector.memset(V[:, :, :, 0:1], 1.0)
                for j in range(TJ):
                    jsl = slice(j * P, (j + 1) * P)
                    for hn in range(TN):
                        hsl = slice(hn * NF, (hn + 1) * NF)
                        pv = stile("pv")
                        for k in range(KC):
                            nc.tensor.matmul(pv[:], lhsT=z[:, k, jsl],
                                             rhs=wv[:, k, hsl],
                                             start=(k == 0), stop=False)
                        nc.tensor.matmul(pv[:], lhsT=onesR[0:1, :],
                                         rhs=extbv[0:1, hsl],
                                         start=False, stop=True)
                        pvh = pv[:].rearrange("p (h d) -> p h d", d=Dh)
                        nc.scalar.activation(
                            out=V[:, j, hn * 8:(hn + 1) * 8, 1:Dh + 1],
                            in_=pvh[:], func=ACTF.Copy, scale=1.0)

                # prefetch late weights into freed arena space
                wp = big("T0", "wp")
                bulk_dma(out=wp[:],
                         in_=wp_d.ap().rearrange("p (k n) -> p k n", k=KC))
                w1 = big("T1", "w1")
                bulk_dma(out=w1[:],
                         in_=w1_d.ap().rearrange("p (k n) -> p k n", k=KC))
                w2 = big("T2", "w2")
                bulk_dma(out=w2[:],
                         in_=w2_d.ap().rearrange("p (k n) -> p k n", k=KC))

                # ---------------- attention ----------------
                if phases < 3:
                    for m in range(KC):
                        bulk_dma(out=out_d[m * P:(m + 1) * P, :],
                                 in_=QT[:, m, :])
                    continue
                attnT = big("T8", "attnT")
                with ExitStack() as S:
                    ep = S.enter_context(tc.tile_pool(name="ep", bufs=8))
                    np_ = S.enter_context(tc.tile_pool(name="np", bufs=3))
                    anp = S.enter_context(tc.tile_pool(name="anp", bufs=3))
                    pend_bc = []    # (pasb0, pasb1, m, tsl) awaiting bcast+mult
                    pend_sh = []    # (a0n, a1n, m, tsl) awaiting shift+copy

                    def emit_bcast_mult():
                        pasb0, pasb1, m_, tsl_ = pend_bc.pop(0)
                        rbp0 = stile("rb0")
                        rbp1 = stile("rb1")
                        nc.tensor.matmul(rbp0[0:Dh + 1, :],
                                         lhsT=onesR[0:1, 0:Dh + 1],
                                         rhs=pasb0[0:1, :],
                                         start=True, stop=True)
                        nc.tensor.matmul(rbp1[0:Dh + 1, :],
                                         lhsT=onesR[0:1, 0:Dh + 1],
                                         rhs=pasb1[0:1, :],
                                         start=True, stop=True)
                        a0n = anp.tile([Dh + 1, NF], BF16, tag="a0n")
                        a1n = anp.tile([Dh + 1, NF], BF16, tag="a1n")
                        nc.vector.tensor_tensor(a0n[:], pasb0[:],
                                                rbp0[0:Dh + 1, :], ALU.mult)
                        nc.vector.tensor_tensor(a1n[:], pasb1[:],
                                                rbp1[0:Dh + 1, :], ALU.mult)
                        pend_sh.append((a0n, a1n, m_, tsl_))

                    def emit_shift():
                        a0n, a1n, m_, tsl_ = pend_sh.pop(0)
                        pshift = stile("psh")
                        nc.tensor.matmul(pshift[0:Dh, :], lhsT=shiftS(),
                                         rhs=a0n[:], start=True, stop=True)
                        nc.tensor.matmul(pshift[Dh:P, :], lhsT=shiftS(),
                                         rhs=a1n[:], start=True, stop=True)
                        nc.scalar.activation(out=attnT[:, m_, tsl_],
                                             in_=pshift[:],
                                             func=ACTF.Copy, scale=1.0)

                    for m in range(KC):
                        h0, h1 = 2 * m, 2 * m + 1
                        for tn in range(TN):
                            tsl = slice(tn * NF, (tn + 1) * NF)
                            i_hi = 4 * (tn + 1)
                            # pass 1: scores + exp (+causal 0/1 mask on E)
                            Es = []
                            for i in range(i_hi):
                                diag = i - 4 * tn
                                d0 = P * max(diag, 0)
                                w = NF - d0
                                ssl = slice(i * P, (i + 1) * P)
                                qsl = slice(tn * NF + d0, (tn + 1) * NF)
                                ps0 = stile("ps0")
                                ps1 = stile("ps1")
                                nc.tensor.matmul(
                                    ps0[:, d0:NF], lhsT=KT[0:64, m, ssl],
                                    rhs=QT[0:64, m, qsl],
                                    start=True, stop=True)
                                nc.tensor.matmul(
                                    ps1[:, d0:NF], lhsT=KT[64:128, m, ssl],
                                    rhs=QT[64:128, m, qsl],
                                    start=True, stop=True)
                                E0 = ep.tile([P, NF], BF16, tag="E0")
                                E1 = ep.tile([P, NF], BF16, tag="E1")
                                nc.scalar.activation(out=E0[:, 0:w],
                                                     in_=ps0[:, d0:NF],
                                                     func=ACTF.Exp,
                                                     scale=Dh ** -0.5)
                                nc.scalar.activation(out=E1[:, 0:w],
                                                     in_=ps1[:, d0:NF],
                                                     func=ACTF.Exp,
                                                     scale=Dh ** -0.5)
                                if diag >= 0:
                                    nc.vector.tensor_tensor(
                                        E0[:, 0:P], E0[:, 0:P], maskS(),
                                        ALU.mult)
                                    nc.vector.tensor_tensor(
                                        E1[:, 0:P], E1[:, 0:P], maskS(),
                                        ALU.mult)
                                Es.append((E0, E1, d0, w))
                            # pass 2: A = V' @ E accumulation
                            pab0 = atile("pa0")
                            pab1 = atile("pa1")
                            for i, (E0, E1, d0, w) in enumerate(Es):
                                nc.tensor.matmul(
                                    pab0[0:Dh + 1, d0:NF],
                                    lhsT=V[:, i, h0, :], rhs=E0[:, 0:w],
                                    start=(i == 0), stop=(i == i_hi - 1))
                                nc.tensor.matmul(
                                    pab1[0:Dh + 1, d0:NF],
                                    lhsT=V[:, i, h1, :], rhs=E1[:, 0:w],
                                    start=(i == 0), stop=(i == i_hi - 1))
                            # evacuate + reciprocal of sums (sum row at p0)
                            pasb0 = np_.tile([Dh + 1, NF], BF16, tag="pasb0")
                            pasb1 = np_.tile([Dh + 1, NF], BF16, tag="pasb1")
                            nc.vector.tensor_copy(out=pasb0[:],
                                                  in_=pab0[0:Dh + 1, :])
                            nc.vector.tensor_copy(out=pasb1[:],
                                                  in_=pab1[0:Dh + 1, :])
                            with nc.allow_low_precision(reason="softmax recip"):
                                nc.vector.reciprocal(pasb0[0:1, :],
                                                     pasb0[0:1, :])
                                nc.vector.reciprocal(pasb1[0:1, :],
                                                     pasb1[0:1, :])
                            pend_bc.append((pasb0, pasb1, m, tsl))
                            # pipelined tail work from earlier iterations
                            if len(pend_bc) >= 2:
                                emit_bcast_mult()
                            if len(pend_sh) >= 2:
                                emit_shift()
                    while pend_bc:
                        emit_bcast_mult()
                    while pend_sh:
                        emit_shift()

                # ---------------- proj + residual -> y^T ----------------
                if phases < 4:
                    for m in range(KC):
                        bulk_dma(out=out_d[m * P:(m + 1) * P, :],
                                 in_=attnT[:, m, :])
                    continue
                yT = big("T3", "yT")
                for tn in range(TN):
                    tsl = slice(tn * NF, (tn + 1) * NF)
                    for m in range(KC):
                        msl = slice(m * P, (m + 1) * P)
                        pp = stile("pp")
                        for k in range(KC):
                            nc.tensor.matmul(pp[:], lhsT=wp[:, k, msl],
                                             rhs=attnT[:, k, tsl],
                                             start=(k == 0), stop=(k == KC - 1))
                        nc.vector.scalar_tensor_tensor(
                            out=yT[:, m, tsl], in0=pp[:],
                            scalar=cols[:, 16 + m:17 + m], in1=xt[:, m, tsl],
                            op0=ALU.add, op1=ALU.add)

                # ---------------- LN2 stats (folded rows) ----------------
                if phases < 5:
                    for m in range(KC):
                        bulk_dma(out=out_d[m * P:(m + 1) * P, :],
                                 in_=yT[:, m, :])
                    continue
                sq2 = big("T8", "sq2")
                for k in range(KC):
                    nc.scalar.activation(out=sq2[:, k, :], in_=yT[:, k, :],
                                         func=ACTF.Square, scale=1.0)
                with ExitStack() as S:
                    rtmp = S.enter_context(tc.tile_pool(name="rtmp2", bufs=2))
                    for tn in range(TN):
                        tsl = slice(tn * NF, (tn + 1) * NF)
                        ps_m = stile("psm2", (1, NF))
                        for k in range(KC):
                            nc.tensor.matmul(ps_m[:], lhsT=onesC[:],
                                             rhs=yT[:, k, tsl],
                                             start=(k == 0), stop=(k == KC - 1))
                        ps_s2 = stile("pss2", (1, NF))
                        for k in range(KC):
                            nc.tensor.matmul(ps_s2[:], lhsT=onesC[:],
                                             rhs=sq2[:, k, tsl],
                                             start=(k == 0), stop=(k == KC - 1))
                        msq = rtmp.tile([1, NF], F32, tag="msq")
                        nc.scalar.activation(out=msq[:], in_=ps_m[:],
                                             func=ACTF.Square, scale=1.0)
                        var = rtmp.tile([1, NF], F32, tag="var")
                        nc.vector.scalar_tensor_tensor(
                            out=var[:], in0=ps_s2[:], scalar=1.0, in1=msq[:],
                            op0=ALU.mult, op1=ALU.subtract)
                        # std2 (bf16); copy to mrows p1 via 1-ch broadcast shift
                        nc.scalar.activation(out=std2row[0:1, tsl], in_=var[:],
                                             func=ACTF.Sqrt,
                                             bias=epsc[0:1, :], scale=1.0)
                        bulk_dma(out=mrows[1:2, tsl],
                                 in_=std2row[0:1, tsl])
                        with nc.allow_low_precision(reason="ln2 rstd bf16"):
                            nc.vector.reciprocal(r2row[0:1, tsl],
                                                 std2row[0:1, tsl])
                        nc.scalar.activation(out=mrows[0:1, tsl], in_=ps_m[:],
                                             func=ACTF.Copy, scale=1.0)
                        bc2 = stile("bc2")
                        nc.tensor.matmul(bc2[:], lhsT=onesR[0:1, :],
                                         rhs=r2row[0:1, tsl],
                                         start=True, stop=True)
                        nc.scalar.activation(out=r2b[:, tsl], in_=bc2[:],
                                             func=ACTF.Copy, scale=1.0)

                # ---------------- MLP fc1 ----------------
                if phases < 6:
                    for m in range(KC):
                        bulk_dma(out=out_d[m * P:(m + 1) * P, :],
                                 in_=yT[:, m, :])
                    continue
                hT = big("T4", "hT")
                for tn in range(TN):
                    tsl = slice(tn * NF, (tn + 1) * NF)
                    for m in range(KC):
                        msl = slice(m * P, (m + 1) * P)
                        ph = stile("ph")
                        for k in range(KC):
                            nc.tensor.matmul(ph[:], lhsT=w1[:, k, msl],
                                             rhs=yT[:, k, tsl],
                                             start=(k == 0), stop=False)
                        nc.tensor.matmul(ph[:], lhsT=ext01[0:2, msl],
                                         rhs=mrows[0:2, tsl],
                                         start=False, stop=True)
                        nc.scalar.activation(out=hT[:, m, tsl], in_=ph[:],
                                             func=ACTF.Relu, scale=1.0)

                # ---------------- MLP fc2 + residual -> out^T ----------------
                if phases < 7:
                    for m in range(KC):
                        bulk_dma(out=out_d[m * P:(m + 1) * P, :],
                                 in_=hT[:, m, :])
                    continue
                osb = big("T5", "osb")
                with ExitStack() as S:
                    otp = S.enter_context(tc.tile_pool(name="otp", bufs=3))
                    for m in range(KC):
                        msl = slice(m * P, (m + 1) * P)
                        for tn in range(TN):
                            tsl = slice(tn * NF, (tn + 1) * NF)
                            po = stile("po")
                            for k in range(KC):
                                nc.tensor.matmul(po[:], lhsT=w2[:, k, msl],
                                                 rhs=hT[:, k, tsl],
                                                 start=(k == 0), stop=False)
                            nc.tensor.matmul(po[:], lhsT=extb2[0:1, msl],
                                             rhs=std2row[0:1, tsl],
                                             start=False, stop=True)
                            tmp = otp.tile([P, NF], BF16, tag="tmp")
                            nc.vector.tensor_tensor(tmp[:], po[:],
                                                    r2b[:, tsl], ALU.mult)
                            nc.vector.tensor_tensor(osb[:, m, tsl], tmp[:],
                                                    yT[:, m, tsl], ALU.add)
                        bulk_dma(out=out_d[m * P:(m + 1) * P, :],
                                 in_=osb[:, m, :])

    nc.compile()
    return nc


def _prep_inputs(inputs):
    """Host-side prep: dtype casts, transposes, LN gain/bias folds."""
    f = np.float32
    bf = ml_dtypes.bfloat16
    x = np.asarray(inputs["x"], dtype=f)                       # [B, T, C]
    g1 = np.asarray(inputs["g1"], dtype=f)
    be1 = np.asarray(inputs["beta1"], dtype=f)
    g2 = np.asarray(inputs["g2"], dtype=f)
    be2 = np.asarray(inputs["beta2"], dtype=f)
    Wq = np.asarray(inputs["Wq"], dtype=f).transpose(1, 0, 2).reshape(C, C)
    Wk = np.asarray(inputs["Wk"], dtype=f).transpose(1, 0, 2).reshape(C, C)
    Wv = np.asarray(inputs["Wv"], dtype=f).transpose(1, 0, 2).reshape(C, C)
    Wp = np.asarray(inputs["Wproj"], dtype=f)
    W1 = np.asarray(inputs["W1"], dtype=f)
    W2 = np.asarray(inputs["W2"], dtype=f)
    b1 = np.asarray(inputs["b1"], dtype=f)
    b2 = np.asarray(inputs["b2"], dtype=f)
    bp = np.asarray(inputs["bproj"], dtype=f)

    w1g = g2[:, None] * W1
    cols = np.zeros((P, 24), f)
    cols[:, 0:8] = (be1 @ Wq).reshape(KC, P).T
    cols[:, 8:16] = (be1 @ Wk).reshape(KC, P).T
    cols[:, 16:24] = bp.reshape(KC, P).T
    ext = np.zeros((4, C), f)
    ext[0] = -np.sum(w1g, axis=0)
    ext[1] = b1 + be2 @ W1
    ext[2] = b2
    ext[3] = be1 @ Wv
    consts = np.zeros((P, 2 * P + 64), f)
    consts[:, 0:P] = np.eye(P, dtype=f)
    consts[:, P:2 * P] = np.where(
        np.arange(P)[:, None] <= np.arange(P)[None, :], 1.0, 0.0)
    # shiftS[i, j] = 1 iff i == j+1: out[j] = in[j+1] (drop sum row 0)
    consts[0:Dh + 1, 2 * P:2 * P + Dh] = np.eye(Dh + 1, Dh, k=-1, dtype=f)

    def shuf(a):
        # [C, X] -> [P, KC*X]: DRAM row p holds chunks k at [k*X:(k+1)*X]
        X = a.shape[1]
        return np.ascontiguousarray(
            a.reshape(KC, P, X).transpose(1, 0, 2).reshape(P, KC * X)
        ).astype(bf)

    common = {
        "wq": shuf(g1[:, None] * Wq),
        "wk": shuf(g1[:, None] * Wk),
        "wv": shuf(g1[:, None] * Wv),
        "wp": shuf(Wp),
        "w1": shuf(w1g),
        "w2": shuf(W2),
        "cols": cols,
        "ext": ext.astype(bf),
        "consts": consts.astype(bf),
    }
    return [{"xt": shuf(np.ascontiguousarray(x[b].T)), **common}
            for b in range(N_CORES)]


def kernel(**inputs) -> np.ndarray:
    if "nc" not in _CACHE:
        _CACHE["nc"] = build_nc()
    nc = _CACHE["nc"]
    in_maps = _prep_inputs(inputs)
    res = run_bass_kernel_spmd(nc, in_maps, list(range(N_CORES)))
    out = np.stack(
        [np.asarray(res.results[b]["out"]).astype(np.float32).T
         for b in range(N_CORES)], axis=0)
    return np.ascontiguousarray(out)


if __name__ == "__main__":
    rng = np.random.default_rng(0)
    demo = {
        "x": rng.standard_normal((B, T, C), dtype=np.float32),
        "Wq": rng.standard_normal((H, C, Dh), dtype=np.float32) * 0.02,
        "Wk": rng.standard_normal((H, C, Dh), dtype=np.float32) * 0.02,
        "Wv": rng.standard_normal((H, C, Dh), dtype=np.float32) * 0.02,
        "Wproj": rng.standard_normal((C, C), dtype=np.float32) * 0.02,
        "bproj": np.zeros(C, np.float32),
        "W1": rng.standard_normal((C, C), dtype=np.float32) * 0.02,
        "b1": np.zeros(C, np.float32),
        "W2": rng.standard_normal((C, C), dtype=np.float32) * 0.02,
        "b2": np.zeros(C, np.float32),
        "g1": np.ones(C, np.float32),
        "beta1": np.zeros(C, np.float32),
        "g2": np.ones(C, np.float32),
        "beta2": np.zeros(C, np.float32),
    }
    y = kernel(**demo)
    print("out", y.shape, y.dtype, float(np.abs(y).max()))



# revision 4
# speedup vs baseline: 1.0660x; 1.0660x over previous
"""Trainium2 Bass kernel for nn_Block_3616362463321 (dense transformer block), v3.

B=8, T=1024, C=1024, H=16, Dh=64. Data-parallel: core b computes batch elem b.

v3 over v2:
- The six CxC GEMMs (Q,K,V,proj,fc1,fc2) run in fp8e4 with DoubleRow perf
  mode: 2 contraction k-tiles per matmul instruction at 2x row rate.
  Weights are host-quantized to fp8 scaled by 2^10; activations quantized on
  device scaled by 2^4 (z via the r1b broadcast scale, attnT via the shift
  evacuation scale, hT via the relu scale, yTq via a tensor_scalar mult).
  Descale by 2^-14 is folded into each existing PSUM-evacuation scale knob.
- bproj moves from the yT scalar_tensor_tensor bias to an extra contraction
  row (the evacuation now multiplies by the descale, which would corrupt an
  additive bias).
- LN square tiles (x^2, y^2) computed on DVE (4x bf16 mode) instead of ACT.
- All bulk DMAs issue from SP/Pool queues; ACT issues none (descriptor
  generation was serializing the Activation engine behind exp/evacuations).
- Scores and AV remain bf16 (scores K=64 gets no DoubleRow benefit).
"""
import sys

sys.path.insert(0, "/opt/trn_rl_repo")

from contextlib import ExitStack, nullcontext

import numpy as np
import ml_dtypes

import concourse.bacc as bacc
import concourse.bass as bass
import concourse.mybir as mybir
import concourse.tile as tile
from concourse.bass_utils import run_bass_kernel_spmd

P = 128
B, T, C, H = 8, 1024, 1024, 16
Dh = C // H            # 64
EPS = 1e-5
NF = 512               # PSUM bank free dim (fp32)
KC = C // P            # 8 chunks of 128
TJ = T // P            # 8 t-blocks of 128
TN = T // NF           # 2 t-halves of 512
F32 = mybir.dt.float32
BF16 = mybir.dt.bfloat16
FP8 = mybir.dt.float8e4
ALU = mybir.AluOpType
ACTF = mybir.ActivationFunctionType
DRM = mybir.MatmulPerfMode.DoubleRow

# fp8 stage flags + scales
FP8_QKV = True
FP8_PROJ = True
FP8_MLP1 = True
FP8_MLP2 = True
SW = 2.0 ** 10         # weight scale (all fp8 weights)
SZ = 2.0 ** 4          # z scale
SA = 2.0 ** 4          # attnT scale
SH = 2.0 ** 4          # hT scale
SY = 2.0 ** 4          # yTq scale

QKV_S = SW * SZ if FP8_QKV else 1.0
PROJ_S = SW * SA if FP8_PROJ else 1.0
MLP1_S = SW * SY if FP8_MLP1 else 1.0
MLP2_S = SW * SH if FP8_MLP2 else 1.0

N_CORES = 8
_CACHE = {}


def build_nc(loop=1, hwloop=0, phases=7):
    nc = bacc.Bacc("TRN2", target_bir_lowering=False, debug=False)

    w_dt = {"wq": FP8 if FP8_QKV else BF16, "wk": FP8 if FP8_QKV else BF16,
            "wv": FP8 if FP8_QKV else BF16, "wp": FP8 if FP8_PROJ else BF16,
            "w1": FP8 if FP8_MLP1 else BF16, "w2": FP8 if FP8_MLP2 else BF16}
    xt_d = nc.dram_tensor("xt", [P, KC * T], BF16, kind="ExternalInput")
    wq_d = nc.dram_tensor("wq", [P, KC * C], w_dt["wq"], kind="ExternalInput")
    wk_d = nc.dram_tensor("wk", [P, KC * C], w_dt["wk"], kind="ExternalInput")
    wv_d = nc.dram_tensor("wv", [P, KC * C], w_dt["wv"], kind="ExternalInput")
    wp_d = nc.dram_tensor("wp", [P, KC * C], w_dt["wp"], kind="ExternalInput")
    w1_d = nc.dram_tensor("w1", [P, KC * C], w_dt["w1"], kind="ExternalInput")
    w2_d = nc.dram_tensor("w2", [P, KC * C], w_dt["w2"], kind="ExternalInput")
    # bias columns, fp32: [:,0:8]=beq  [:,8:16]=bek
    cols_d = nc.dram_tensor("cols", [P, 24], F32, kind="ExternalInput")
    # extra contraction rows, bf16 (pre-scaled to their stage's PSUM scale):
    # 0:-c1  1:b1'  2:b2  3:bev  4:bproj
    ext_d = nc.dram_tensor("ext", [5, C], BF16, kind="ExternalInput")
    # [:,0:128]=ident [:,128:256]=causal 0/1 tri [:,256:320]=row-shift
    consts_d = nc.dram_tensor("consts", [P, 2 * P + 64], BF16,
                              kind="ExternalInput")
    out_d = nc.dram_tensor("out", [C, T], BF16, kind="ExternalOutput")

    with tile.TileContext(nc) as tc, ExitStack() as ES:
        singles = ES.enter_context(tc.tile_pool(name="singles", bufs=1))
        arena = ES.enter_context(tc.tile_pool(name="arena", bufs=1))
        psum = ES.enter_context(tc.tile_pool(name="psum", bufs=1, space="PSUM"))

        _si = [0]
        _ai = [0]

        def stile(nm="s", shape=(P, NF)):
            t = psum.tile(list(shape), F32, tag=f"S{_si[0] % 4}",
                          name=f"{nm}{_si[0]}")
            _si[0] += 1
            return t

        def atile(nm="a"):
            t = psum.tile([P, NF], F32, tag=f"A{_ai[0] % 4}", name=f"{nm}{_ai[0]}")
            _ai[0] += 1
            return t

        def big(tag, nm, shape=(P, KC, T), dtype=BF16):
            return arena.tile(list(shape), dtype, tag=tag, name=nm)

        _dq = [0]

        def bulk_dma(out, in_):
            eng = nc.sync if _dq[0] % 2 == 0 else nc.gpsimd
            _dq[0] += 1
            eng.dma_start(out=out, in_=in_)

        def gemm_acc(ps, w, rhs_t, msl, tsl, is_fp8, tail=None):
            """Accumulate ps += w[:, :, msl]^T @ rhs_t[:, :, tsl] over KC
            k-chunks (DoubleRow pairs when fp8), then optional extra rows
            tail=(lhsT, rhs) closing the accumulation group."""
            if is_fp8:
                for k in range(0, KC, 2):
                    nc.tensor.matmul(ps[:], lhsT=w[:, k:k + 2, msl],
                                     rhs=rhs_t[:, k:k + 2, tsl],
                                     start=(k == 0),
                                     stop=(tail is None and k == KC - 2),
                                     perf_mode=DRM)
            else:
                for k in range(KC):
                    nc.tensor.matmul(ps[:], lhsT=w[:, k, msl],
                                     rhs=rhs_t[:, k, tsl],
                                     start=(k == 0),
                                     stop=(tail is None and k == KC - 1))
            if tail is not None:
                lt, rt = tail
                nc.tensor.matmul(ps[:], lhsT=lt, rhs=rt,
                                 start=False, stop=True)

        # ---- constants / small tiles ----
        consts = singles.tile([P, 2 * P + 64], BF16)
        nc.sync.dma_start(out=consts[:], in_=consts_d[:, :])
        cols = singles.tile([P, 24], F32)
        nc.sync.dma_start(out=cols[:], in_=cols_d[:, :])
        ext01 = singles.tile([2, C], BF16)   # rows: -c1, b1'
        nc.sync.dma_start(out=ext01[:], in_=ext_d[0:2, :])
        extb2 = singles.tile([1, C], BF16)   # b2
        nc.sync.dma_start(out=extb2[:], in_=ext_d[2:3, :])
        extbv = singles.tile([1, C], BF16)   # bev
        nc.gpsimd.dma_start(out=extbv[:], in_=ext_d[3:4, :])
        extbp = singles.tile([1, C], BF16)   # bproj
        nc.gpsimd.dma_start(out=extbp[:], in_=ext_d[4:5, :])
        onesC = singles.tile([P, 1], BF16)
        nc.vector.memset(onesC[:], 1.0 / C)
        onesR = singles.tile([1, P], BF16)
        nc.vector.memset(onesR[:], 1.0)
        onesT = singles.tile([1, T], BF16)
        nc.vector.memset(onesT[:], 1.0)
        epsc = singles.tile([P, 1], F32)
        nc.vector.memset(epsc[:], EPS)

        # LN stat rows / broadcast tiles
        m1row = singles.tile([1, T], BF16)
        r1row = singles.tile([1, T], BF16)
        mrows = singles.tile([2, T], BF16)   # p0: m2, p1: std2
        std2row = singles.tile([1, T], BF16)
        r2row = singles.tile([1, T], BF16)
        r2b = singles.tile([P, T], BF16)

        def ident():
            return consts[:, 0:P]

        def maskS():
            return consts[:, P:2 * P]

        def shiftS():
            return consts[0:Dh + 1, 2 * P:2 * P + Dh]

        with (tc.For_i(0, hwloop, 1) if hwloop else nullcontext()):
            for _it in range(loop):
                # ---------------- load x^T + first weights ----------------
                xt = big("T5", "xt")
                nc.sync.dma_start(
                    out=xt[:], in_=xt_d.ap().rearrange("p (k t) -> p k t", k=KC))
                wq = big("T2", "wq", dtype=w_dt["wq"])
                bulk_dma(out=wq[:],
                         in_=wq_d.ap().rearrange("p (k n) -> p k n", k=KC))
                wk = big("T3", "wk", dtype=w_dt["wk"])
                bulk_dma(out=wk[:],
                         in_=wk_d.ap().rearrange("p (k n) -> p k n", k=KC))
                wv = big("T4", "wv", dtype=w_dt["wv"])
                bulk_dma(out=wv[:],
                         in_=wv_d.ap().rearrange("p (k n) -> p k n", k=KC))

                # ---------------- LN1 stats ----------------
                LN1 = ExitStack()
                ln1bc = LN1.enter_context(tc.tile_pool(name="ln1bc", bufs=1))
                m1b = ln1bc.tile([P, T], BF16, tag="m1b", name="m1b")
                r1b = ln1bc.tile([P, T], BF16, tag="r1b", name="r1b")
                sq = big("T0", "sq")
                for k in range(KC):
                    nc.vector.tensor_tensor(sq[:, k, :], xt[:, k, :],
                                            xt[:, k, :], ALU.mult)
                with ExitStack() as S:
                    rtmp = S.enter_context(tc.tile_pool(name="rtmp", bufs=2))
                    for tn in range(TN):
                        tsl = slice(tn * NF, (tn + 1) * NF)
                        ps_m = stile("psm", (1, NF))
                        for k in range(KC):
                            nc.tensor.matmul(ps_m[:], lhsT=onesC[:],
                                             rhs=xt[:, k, tsl],
                                             start=(k == 0), stop=(k == KC - 1))
                        ps_s2 = stile("pss", (1, NF))
                        for k in range(KC):
                            nc.tensor.matmul(ps_s2[:], lhsT=onesC[:],
                                             rhs=sq[:, k, tsl],
                                             start=(k == 0), stop=(k == KC - 1))
                        msq = rtmp.tile([1, NF], F32, tag="msq")
                        nc.scalar.activation(out=msq[:], in_=ps_m[:],
                                             func=ACTF.Square, scale=1.0)
                        var = rtmp.tile([1, NF], F32, tag="var")
                        nc.vector.scalar_tensor_tensor(
                            out=var[:], in0=ps_s2[:], scalar=1.0, in1=msq[:],
                            op0=ALU.mult, op1=ALU.subtract)
                        stdr = rtmp.tile([1, NF], F32, tag="stdr")
                        nc.scalar.activation(out=stdr[:], in_=var[:],
                                             func=ACTF.Sqrt,
                                             bias=epsc[0:1, :], scale=1.0)
                        with nc.allow_low_precision(reason="ln rstd bf16"):
                            nc.vector.reciprocal(r1row[0:1, tsl], stdr[:])
                        nc.scalar.activation(out=m1row[0:1, tsl], in_=ps_m[:],
                                             func=ACTF.Copy, scale=1.0)
                        bc0 = stile("bc0")
                        nc.tensor.matmul(bc0[:], lhsT=onesR[0:1, :],
                                         rhs=m1row[0:1, tsl],
                                         start=True, stop=True)
                        nc.scalar.activation(out=m1b[:, tsl], in_=bc0[:],
                                             func=ACTF.Copy, scale=1.0)
                        bc1 = stile("bc1")
                        nc.tensor.matmul(bc1[:], lhsT=onesR[0:1, :],
                                         rhs=r1row[0:1, tsl],
                                         start=True, stop=True)
                        # z-quant scale folded into the rstd broadcast
                        nc.scalar.activation(out=r1b[:, tsl], in_=bc1[:],
                                             func=ACTF.Copy,
                                             scale=SZ if FP8_QKV else 1.0)

                # ---------------- z = (x^T - m1)*r1*SZ ----------------
                zsub = big("T9", "zsub")
                z = big("T1", "z", dtype=FP8 if FP8_QKV else BF16)
                for k in range(KC):
                    nc.vector.tensor_tensor(zsub[:, k, :], xt[:, k, :],
                                            m1b[:, :], ALU.subtract)
                    nc.vector.tensor_tensor(z[:, k, :], zsub[:, k, :],
                                            r1b[:, :], ALU.mult)
                LN1.close()

                # ---------------- QKV ----------------
                if phases < 2:
                    for m in range(KC):
                        bulk_dma(out=out_d[m * P:(m + 1) * P, :],
                                 in_=zsub[:, m, :])
                    continue
                QT = big("T6", "QT")
                KT = big("T7", "KT")
                for tn in range(TN):
                    tsl = slice(tn * NF, (tn + 1) * NF)
                    for m in range(KC):
                        msl = slice(m * P, (m + 1) * P)
                        pq = stile("pq")
                        gemm_acc(pq, wq, z, msl, tsl, FP8_QKV)
                        nc.scalar.activation(out=QT[:, m, tsl], in_=pq[:],
                                             func=ACTF.Identity,
                                             bias=cols[:, m:m + 1],
                                             scale=1.0 / QKV_S)
                        pk = stile("pk")
                        gemm_acc(pk, wk, z, msl, tsl, FP8_QKV)
                        nc.scalar.activation(out=KT[:, m, tsl], in_=pk[:],
                                             func=ACTF.Identity,
                                             bias=cols[:, 8 + m:9 + m],
                                             scale=1.0 / QKV_S)

                # V natural [t(part), j, h, 65]; ones col first
                V = big("TV", "V", (P, TJ, H, Dh + 1))
                nc.vector.memset(V[:, :, :, 0:1], 1.0)
                for j in range(TJ):
                    jsl = slice(j * P, (j + 1) * P)
                    for hn in range(TN):
                        hsl = slice(hn * NF, (hn + 1) * NF)
                        pv = stile("pv")
                        if FP8_QKV:
                            for k in range(0, KC, 2):
                                nc.tensor.matmul(pv[:], lhsT=z[:, k:k + 2, jsl],
                                                 rhs=wv[:, k:k + 2, hsl],
                                                 start=(k == 0), stop=False,
                                                 perf_mode=DRM)
                        else:
                            for k in range(KC):
                                nc.tensor.matmul(pv[:], lhsT=z[:, k, jsl],
                                                 rhs=wv[:, k, hsl],
                                                 start=(k == 0), stop=False)
                        nc.tensor.matmul(pv[:], lhsT=onesR[0:1, :],
                                         rhs=extbv[0:1, hsl],
                                         start=False, stop=True)
                        pvh = pv[:].rearrange("p (h d) -> p h d", d=Dh)
                        nc.scalar.activation(
                            out=V[:, j, hn * 8:(hn + 1) * 8, 1:Dh + 1],
                            in_=pvh[:], func=ACTF.Copy, scale=1.0 / QKV_S)

                # prefetch late weights into freed arena space
                wp = big("T0", "wp", dtype=w_dt["wp"])
                bulk_dma(out=wp[:],
                         in_=wp_d.ap().rearrange("p (k n) -> p k n", k=KC))
                w1 = big("T1", "w1", dtype=w_dt["w1"])
                bulk_dma(out=w1[:],
                         in_=w1_d.ap().rearrange("p (k n) -> p k n", k=KC))
                w2 = big("T2", "w2", dtype=w_dt["w2"])
                bulk_dma(out=w2[:],
                         in_=w2_d.ap().rearrange("p (k n) -> p k n", k=KC))

                # ---------------- attention ----------------
                if phases < 3:
                    for m in range(KC):
                        bulk_dma(out=out_d[m * P:(m + 1) * P, :],
                                 in_=QT[:, m, :])
                    continue
                attnT = big("T8", "attnT", dtype=FP8 if FP8_PROJ else BF16)
                with ExitStack() as S:
                    ep = S.enter_context(tc.tile_pool(name="ep", bufs=8))
                    np_ = S.enter_context(tc.tile_pool(name="np", bufs=3))
                    anp = S.enter_context(tc.tile_pool(name="anp", bufs=3))
                    pend_bc = []    # (pasb0, pasb1, m, tsl) awaiting bcast+mult
                    pend_sh = []    # (a0n, a1n, m, tsl) awaiting shift+copy

                    def emit_bcast_mult():
                        pasb0, pasb1, m_, tsl_ = pend_bc.pop(0)
                        rbp0 = stile("rb0")
                        rbp1 = stile("rb1")
                        nc.tensor.matmul(rbp0[0:Dh + 1, :],
                                         lhsT=onesR[0:1, 0:Dh + 1],
                                         rhs=pasb0[0:1, :],
                                         start=True, stop=True)
                        nc.tensor.matmul(rbp1[0:Dh + 1, :],
                                         lhsT=onesR[0:1, 0:Dh + 1],
                                         rhs=pasb1[0:1, :],
                                         start=True, stop=True)
                        a0n = anp.tile([Dh + 1, NF], BF16, tag="a0n")
                        a1n = anp.tile([Dh + 1, NF], BF16, tag="a1n")
                        nc.vector.tensor_tensor(a0n[:], pasb0[:],
                                                rbp0[0:Dh + 1, :], ALU.mult)
                        nc.vector.tensor_tensor(a1n[:], pasb1[:],
                                                rbp1[0:Dh + 1, :], ALU.mult)
                        pend_sh.append((a0n, a1n, m_, tsl_))

                    def emit_shift():
                        a0n, a1n, m_, tsl_ = pend_sh.pop(0)
                        pshift = stile("psh")
                        nc.tensor.matmul(pshift[0:Dh, :], lhsT=shiftS(),
                                         rhs=a0n[:], start=True, stop=True)
                        nc.tensor.matmul(pshift[Dh:P, :], lhsT=shiftS(),
                                         rhs=a1n[:], start=True, stop=True)
                        nc.scalar.activation(out=attnT[:, m_, tsl_],
                                             in_=pshift[:],
                                             func=ACTF.Copy,
                                             scale=SA if FP8_PROJ else 1.0)

                    for m in range(KC):
                        h0, h1 = 2 * m, 2 * m + 1
                        for tn in range(TN):
                            tsl = slice(tn * NF, (tn + 1) * NF)
                            i_hi = 4 * (tn + 1)
                            # pass 1: scores + exp (+causal 0/1 mask on E)
                            Es = []
                            for i in range(i_hi):
                                diag = i - 4 * tn
                                d0 = P * max(diag, 0)
                                w = NF - d0
                                ssl = slice(i * P, (i + 1) * P)
                                qsl = slice(tn * NF + d0, (tn + 1) * NF)
                                ps0 = stile("ps0")
                                ps1 = stile("ps1")
                                nc.tensor.matmul(
                                    ps0[:, d0:NF], lhsT=KT[0:64, m, ssl],
                                    rhs=QT[0:64, m, qsl],
                                    start=True, stop=True)
                                nc.tensor.matmul(
                                    ps1[:, d0:NF], lhsT=KT[64:128, m, ssl],
                                    rhs=QT[64:128, m, qsl],
                                    start=True, stop=True)
                                E0 = ep.tile([P, NF], BF16, tag="E0")
                                E1 = ep.tile([P, NF], BF16, tag="E1")
                                nc.scalar.activation(out=E0[:, 0:w],
                                                     in_=ps0[:, d0:NF],
                                                     func=ACTF.Exp,
                                                     scale=Dh ** -0.5)
                                nc.scalar.activation(out=E1[:, 0:w],
                                                     in_=ps1[:, d0:NF],
                                                     func=ACTF.Exp,
                                                     scale=Dh ** -0.5)
                                if diag >= 0:
                                    nc.vector.tensor_tensor(
                                        E0[:, 0:P], E0[:, 0:P], maskS(),
                                        ALU.mult)
                                    nc.vector.tensor_tensor(
                                        E1[:, 0:P], E1[:, 0:P], maskS(),
                                        ALU.mult)
                                Es.append((E0, E1, d0, w))
                            # pass 2: A = V' @ E accumulation
                            pab0 = atile("pa0")
                            pab1 = atile("pa1")
                            for i, (E0, E1, d0, w) in enumerate(Es):
                                nc.tensor.matmul(
                                    pab0[0:Dh + 1, d0:NF],
                                    lhsT=V[:, i, h0, :], rhs=E0[:, 0:w],
                                    start=(i == 0), stop=(i == i_hi - 1))
                                nc.tensor.matmul(
                                    pab1[0:Dh + 1, d0:NF],
                                    lhsT=V[:, i, h1, :], rhs=E1[:, 0:w],
                                    start=(i == 0), stop=(i == i_hi - 1))
                            # evacuate + reciprocal of sums (sum row at p0)
                            pasb0 = np_.tile([Dh + 1, NF], BF16, tag="pasb0")
                            pasb1 = np_.tile([Dh + 1, NF], BF16, tag="pasb1")
                            nc.vector.tensor_copy(out=pasb0[:],
                                                  in_=pab0[0:Dh + 1, :])
                            nc.vector.tensor_copy(out=pasb1[:],
                                                  in_=pab1[0:Dh + 1, :])
                            with nc.allow_low_precision(reason="softmax recip"):
                                nc.vector.reciprocal(pasb0[0:1, :],
                                                     pasb0[0:1, :])
                                nc.vector.reciprocal(pasb1[0:1, :],
                                                     pasb1[0:1, :])
                            pend_bc.append((pasb0, pasb1, m, tsl))
                            # pipelined tail work from earlier iterations
                            if len(pend_bc) >= 2:
                                emit_bcast_mult()
                            if len(pend_sh) >= 2:
                                emit_shift()
                    while pend_bc:
                        emit_bcast_mult()
                    while pend_sh:
                        emit_shift()

                # ---------------- proj + residual -> y^T ----------------
                if phases < 4:
                    for m in range(KC):
                        bulk_dma(out=out_d[m * P:(m + 1) * P, :],
                                 in_=QT[:, m, :])
                    continue
                yT = big("T3", "yT")
                for tn in range(TN):
                    tsl = slice(tn * NF, (tn + 1) * NF)
                    for m in range(KC):
                        msl = slice(m * P, (m + 1) * P)
                        pp = stile("pp")
                        gemm_acc(pp, wp, attnT, msl, tsl, FP8_PROJ,
                                 tail=(extbp[0:1, msl], onesT[0:1, tsl]))
                        nc.vector.scalar_tensor_tensor(
                            out=yT[:, m, tsl], in0=pp[:],
                            scalar=1.0 / PROJ_S, in1=xt[:, m, tsl],
                            op0=ALU.mult, op1=ALU.add)

                # ---------------- LN2 stats (folded rows) ----------------
                if phases < 5:
                    for m in range(KC):
                        bulk_dma(out=out_d[m * P:(m + 1) * P, :],
                                 in_=yT[:, m, :])
                    continue
                sq2 = big("T8", "sq2")
                for k in range(KC):
                    nc.vector.tensor_tensor(sq2[:, k, :], yT[:, k, :],
                                            yT[:, k, :], ALU.mult)
                with ExitStack() as S:
                    rtmp = S.enter_context(tc.tile_pool(name="rtmp2", bufs=2))
                    for tn in range(TN):
                        tsl = slice(tn * NF, (tn + 1) * NF)
                        ps_m = stile("psm2", (1, NF))
                        for k in range(KC):
                            nc.tensor.matmul(ps_m[:], lhsT=onesC[:],
                                             rhs=yT[:, k, tsl],
                                             start=(k == 0), stop=(k == KC - 1))
                        ps_s2 = stile("pss2", (1, NF))
                        for k in range(KC):
                            nc.tensor.matmul(ps_s2[:], lhsT=onesC[:],
                                             rhs=sq2[:, k, tsl],
                                             start=(k == 0), stop=(k == KC - 1))
                        msq = rtmp.tile([1, NF], F32, tag="msq")
                        nc.scalar.activation(out=msq[:], in_=ps_m[:],
                                             func=ACTF.Square, scale=1.0)
                        var = rtmp.tile([1, NF], F32, tag="var")
                        nc.vector.scalar_tensor_tensor(
                            out=var[:], in0=ps_s2[:], scalar=1.0, in1=msq[:],
                            op0=ALU.mult, op1=ALU.subtract)
                        # std2 (bf16); copy to mrows p1 via 1-ch broadcast shift
                        nc.scalar.activation(out=std2row[0:1, tsl], in_=var[:],
                                             func=ACTF.Sqrt,
                                             bias=epsc[0:1, :], scale=1.0)
                        nc.gpsimd.dma_start(out=mrows[1:2, tsl],
                                          in_=std2row[0:1, tsl])
                        with nc.allow_low_precision(reason="ln2 rstd bf16"):
                            nc.vector.reciprocal(r2row[0:1, tsl],
                                                 std2row[0:1, tsl])
                        nc.scalar.activation(out=mrows[0:1, tsl], in_=ps_m[:],
                                             func=ACTF.Copy, scale=1.0)
                        bc2 = stile("bc2")
                        nc.tensor.matmul(bc2[:], lhsT=onesR[0:1, :],
                                         rhs=r2row[0:1, tsl],
                                         start=True, stop=True)
                        # fc2 descale folded into the rstd broadcast
                        nc.scalar.activation(out=r2b[:, tsl], in_=bc2[:],
                                             func=ACTF.Copy,
                                             scale=1.0 / MLP2_S)

                # ---------------- MLP fc1 ----------------
                if phases < 6:
                    for m in range(KC):
                        bulk_dma(out=out_d[m * P:(m + 1) * P, :],
                                 in_=yT[:, m, :])
                    continue
                if FP8_MLP1:
                    yq = big("T9", "yq", dtype=FP8)
                    for k in range(KC):
                        nc.vector.tensor_scalar_mul(yq[:, k, :], yT[:, k, :],
                                                    SY)
                else:
                    yq = yT
                hT = big("T4", "hT", dtype=FP8 if FP8_MLP2 else BF16)
                relu_s = (SH if FP8_MLP2 else 1.0) / MLP1_S
                for tn in range(TN):
                    tsl = slice(tn * NF, (tn + 1) * NF)
                    for m in range(KC):
                        msl = slice(m * P, (m + 1) * P)
                        ph = stile("ph")
                        gemm_acc(ph, w1, yq, msl, tsl, FP8_MLP1,
                                 tail=(ext01[0:2, msl], mrows[0:2, tsl]))
                        nc.scalar.activation(out=hT[:, m, tsl], in_=ph[:],
                                             func=ACTF.Relu, scale=relu_s)

                # ---------------- MLP fc2 + residual -> out^T ----------------
                if phases < 7:
                    for m in range(KC):
                        bulk_dma(out=out_d[m * P:(m + 1) * P, :],
                                 in_=hT[:, m, :])
                    continue
                osb = big("T5", "osb")
                with ExitStack() as S:
                    otp = S.enter_context(tc.tile_pool(name="otp", bufs=3))
                    for m in range(KC):
                        msl = slice(m * P, (m + 1) * P)
                        for tn in range(TN):
                            tsl = slice(tn * NF, (tn + 1) * NF)
                            po = stile("po")
                            gemm_acc(po, w2, hT, msl, tsl, FP8_MLP2,
                                     tail=(extb2[0:1, msl], std2row[0:1, tsl]))
                            tmp = otp.tile([P, NF], BF16, tag="tmp")
                            nc.vector.tensor_tensor(tmp[:], po[:],
                                                    r2b[:, tsl], ALU.mult)
                            nc.vector.tensor_tensor(osb[:, m, tsl], tmp[:],
                                                    yT[:, m, tsl], ALU.add)
                        bulk_dma(out=out_d[m * P:(m + 1) * P, :],
                                 in_=osb[:, m, :])

    nc.compile()
    return nc


def _prep_inputs(inputs):
    """Host-side prep: dtype casts, transposes, LN gain/bias folds, fp8."""
    f = np.float32
    bf = ml_dtypes.bfloat16
    f8 = ml_dtypes.float8_e4m3
    x = np.asarray(inputs["x"], dtype=f)                       # [B, T, C]
    g1 = np.asarray(inputs["g1"], dtype=f)
    be1 = np.asarray(inputs["beta1"], dtype=f)
    g2 = np.asarray(inputs["g2"], dtype=f)
    be2 = np.asarray(inputs["beta2"], dtype=f)
    Wq = np.asarray(inputs["Wq"], dtype=f).transpose(1, 0, 2).reshape(C, C)
    Wk = np.asarray(inputs["Wk"], dtype=f).transpose(1, 0, 2).reshape(C, C)
    Wv = np.asarray(inputs["Wv"], dtype=f).transpose(1, 0, 2).reshape(C, C)
    Wp = np.asarray(inputs["Wproj"], dtype=f)
    W1 = np.asarray(inputs["W1"], dtype=f)
    W2 = np.asarray(inputs["W2"], dtype=f)
    b1 = np.asarray(inputs["b1"], dtype=f)
    b2 = np.asarray(inputs["b2"], dtype=f)
    bp = np.asarray(inputs["bproj"], dtype=f)

    def shuf(a, dt):
        # [C, X] -> [P, KC*X]: DRAM row p holds chunks k at [k*X:(k+1)*X]
        X = a.shape[1]
        return np.ascontiguousarray(
            a.reshape(KC, P, X).transpose(1, 0, 2).reshape(P, KC * X)
        ).astype(dt)

    def wprep(a, is_fp8):
        if is_fp8:
            return shuf(np.clip(a * SW, -240, 240), f8)
        return shuf(a, bf)

    w1g = g2[:, None] * W1
    w1_dev = wprep(w1g, FP8_MLP1)
    # -c1 from the device-visible (quantized) w1 for exact mean cancellation
    if FP8_MLP1:
        w1_eff = w1_dev.astype(f).reshape(P, KC, C).transpose(1, 0, 2) \
            .reshape(C, C) / SW
    else:
        w1_eff = w1_dev.astype(f).reshape(P, KC, C).transpose(1, 0, 2) \
            .reshape(C, C)

    cols = np.zeros((P, 24), f)
    cols[:, 0:8] = (be1 @ Wq).reshape(KC, P).T
    cols[:, 8:16] = (be1 @ Wk).reshape(KC, P).T
    ext = np.zeros((5, C), f)
    ext[0] = -np.sum(w1_eff, axis=0) * MLP1_S
    ext[1] = (b1 + be2 @ W1) * MLP1_S
    ext[2] = b2 * MLP2_S
    ext[3] = (be1 @ Wv) * QKV_S
    ext[4] = bp * PROJ_S
    consts = np.zeros((P, 2 * P + 64), f)
    consts[:, 0:P] = np.eye(P, dtype=f)
    consts[:, P:2 * P] = np.where(
        np.arange(P)[:, None] <= np.arange(P)[None, :], 1.0, 0.0)
    # shiftS[i, j] = 1 iff i == j+1: out[j] = in[j+1] (drop sum row 0)
    consts[0:Dh + 1, 2 * P:2 * P + Dh] = np.eye(Dh + 1, Dh, k=-1, dtype=f)

    common = {
        "wq": wprep(g1[:, None] * Wq, FP8_QKV),
        "wk": wprep(g1[:, None] * Wk, FP8_QKV),
        "wv": wprep(g1[:, None] * Wv, FP8_QKV),
        "wp": wprep(Wp, FP8_PROJ),
        "w1": w1_dev,
        "w2": wprep(W2, FP8_MLP2),
        "cols": cols,
        "ext": ext.astype(bf),
        "consts": consts.astype(bf),
    }
    return [{"xt": shuf(np.ascontiguousarray(x[b].T), bf), **common}
            for b in range(N_CORES)]


def kernel(**inputs) -> np.ndarray:
    if "nc" not in _CACHE:
        _CACHE["nc"] = build_nc()
    nc = _CACHE["nc"]
    in_maps = _prep_inputs(inputs)
    res = run_bass_kernel_spmd(nc, in_maps, list(range(N_CORES)))
    out = np.stack(
        [np.asarray(res.results[b]["out"]).astype(np.float32).T
         for b in range(N_CORES)], axis=0)
    return np.ascontiguousarray(out)


if __name__ == "__main__":
    rng = np.random.default_rng(0)
    demo = {
        "x": rng.standard_normal((B, T, C), dtype=np.float32),
        "Wq": rng.standard_normal((H, C, Dh), dtype=np.float32) * 0.02,
        "Wk": rng.standard_normal((H, C, Dh), dtype=np.float32) * 0.02,
        "Wv": rng.standard_normal((H, C, Dh), dtype=np.float32) * 0.02,
        "Wproj": rng.standard_normal((C, C), dtype=np.float32) * 0.02,
        "bproj": np.zeros(C, np.float32),
        "W1": rng.standard_normal((C, C), dtype=np.float32) * 0.02,
        "b1": np.zeros(C, np.float32),
        "W2": rng.standard_normal((C, C), dtype=np.float32) * 0.02,
        "b2": np.zeros(C, np.float32),
        "g1": np.ones(C, np.float32),
        "beta1": np.zeros(C, np.float32),
        "g2": np.ones(C, np.float32),
        "beta2": np.zeros(C, np.float32),
    }
    y = kernel(**demo)
    print("out", y.shape, y.dtype, float(np.abs(y).max()))


# revision 14
# speedup vs baseline: 1.4890x; 1.3967x over previous
"""Trainium2 Bass kernel for nn_Block_3616362463321 (dense transformer block), v3.

B=8, T=1024, C=1024, H=16, Dh=64. Data-parallel: core b computes batch elem b.

v3 over v2:
- The six CxC GEMMs (Q,K,V,proj,fc1,fc2) run in fp8e4 with DoubleRow perf
  mode: 2 contraction k-tiles per matmul instruction at 2x row rate.
  Weights are host-quantized to fp8 scaled by 2^10; activations quantized on
  device scaled by 2^4 (z via the r1b broadcast scale, attnT via the shift
  evacuation scale, hT via the relu scale, yTq via a tensor_scalar mult).
  Descale by 2^-14 is folded into each existing PSUM-evacuation scale knob.
- bproj moves from the yT scalar_tensor_tensor bias to an extra contraction
  row (the evacuation now multiplies by the descale, which would corrupt an
  additive bias).
- LN square tiles (x^2, y^2) computed on DVE (4x bf16 mode) instead of ACT.
- All bulk DMAs issue from SP/Pool queues; ACT issues none (descriptor
  generation was serializing the Activation engine behind exp/evacuations).
- Scores and AV remain bf16 (scores K=64 gets no DoubleRow benefit).
"""
import sys

sys.path.insert(0, "/opt/trn_rl_repo")

from contextlib import ExitStack, nullcontext

import numpy as np
import ml_dtypes

import concourse.bacc as bacc
import concourse.bass as bass
import concourse.mybir as mybir
import concourse.tile as tile
from concourse.bass_utils import run_bass_kernel_spmd

P = 128
B, T, C, H = 8, 1024, 1024, 16
Dh = C // H            # 64
EPS = 1e-5
NF = 512               # PSUM bank free dim (fp32)
KC = C // P            # 8 chunks of 128
TJ = T // P            # 8 t-blocks of 128
TN = T // NF           # 2 t-halves of 512
F32 = mybir.dt.float32
BF16 = mybir.dt.bfloat16
FP8 = mybir.dt.float8e4
ALU = mybir.AluOpType
ACTF = mybir.ActivationFunctionType
DRM = mybir.MatmulPerfMode.DoubleRow

# fp8 stage flags + scales
FP8_QKV = True
FP8_PROJ = True
FP8_MLP1 = True
FP8_MLP2 = True
SW = 2.0 ** 10         # weight scale (all fp8 weights)
SZ = 2.0 ** 4          # z scale
SA = 2.0 ** 4          # attnT scale
SH = 2.0 ** 4          # hT scale
SY = 2.0 ** 4          # yTq scale

QKV_S = SW * SZ if FP8_QKV else 1.0
PROJ_S = SW * SA if FP8_PROJ else 1.0
MLP1_S = SW * SY if FP8_MLP1 else 1.0
MLP2_S = SW * SH if FP8_MLP2 else 1.0

N_CORES = 8
_CACHE = {}


def build_nc(loop=1, hwloop=0, phases=7):
    nc = bacc.Bacc("TRN2", target_bir_lowering=False, debug=False)

    w_dt = {"wq": FP8 if FP8_QKV else BF16, "wk": FP8 if FP8_QKV else BF16,
            "wv": FP8 if FP8_QKV else BF16, "wp": FP8 if FP8_PROJ else BF16,
            "w1": FP8 if FP8_MLP1 else BF16, "w2": FP8 if FP8_MLP2 else BF16}
    xt_d = nc.dram_tensor("xt", [P, KC * T], BF16, kind="ExternalInput")
    wq_d = nc.dram_tensor("wq", [P, KC * C], w_dt["wq"], kind="ExternalInput")
    wk_d = nc.dram_tensor("wk", [P, KC * C], w_dt["wk"], kind="ExternalInput")
    wv_d = nc.dram_tensor("wv", [P, KC * C], w_dt["wv"], kind="ExternalInput")
    wp_d = nc.dram_tensor("wp", [P, KC * C], w_dt["wp"], kind="ExternalInput")
    w1_d = nc.dram_tensor("w1", [P, KC * C], w_dt["w1"], kind="ExternalInput")
    w2_d = nc.dram_tensor("w2", [P, KC * C], w_dt["w2"], kind="ExternalInput")
    # bias columns, fp32: [:,0:8]=beq  [:,8:16]=bek
    cols_d = nc.dram_tensor("cols", [P, 24], F32, kind="ExternalInput")
    # extra contraction rows, bf16 (pre-scaled to their stage's PSUM scale):
    # 0:-c1  1:b1'  2:b2  3:bev  4:bproj
    ext_d = nc.dram_tensor("ext", [5, C], BF16, kind="ExternalInput")
    # [:,0:128]=ident [:,128:256]=causal 0/1 tri [:,256:320]=row-shift
    consts_d = nc.dram_tensor("consts", [P, 2 * P + 64], BF16,
                              kind="ExternalInput")
    out_d = nc.dram_tensor("out", [C, T], BF16, kind="ExternalOutput")

    with tile.TileContext(nc) as tc, ExitStack() as ES:
        singles = ES.enter_context(tc.tile_pool(name="singles", bufs=1))
        arena = ES.enter_context(tc.tile_pool(name="arena", bufs=1))
        psum = ES.enter_context(tc.tile_pool(name="psum", bufs=1, space="PSUM"))

        _si = [0]
        _ai = [0]

        def stile(nm="s", shape=(P, NF)):
            t = psum.tile(list(shape), F32, tag=f"S{_si[0] % 3}",
                          name=f"{nm}{_si[0]}")
            _si[0] += 1
            return t

        def atile(nm="a"):
            t = psum.tile([P, NF], F32, tag=f"A{_ai[0] % 3}", name=f"{nm}{_ai[0]}")
            _ai[0] += 1
            return t

        def ntile(tag, nm):
            return psum.tile([P, NF], F32, tag=tag, name=nm)

        def big(tag, nm, shape=(P, KC, T), dtype=BF16):
            return arena.tile(list(shape), dtype, tag=tag, name=nm)

        _dq = [0]

        def bulk_dma(out, in_):
            eng = nc.sync if _dq[0] % 2 == 0 else nc.gpsimd
            _dq[0] += 1
            eng.dma_start(out=out, in_=in_)

        def gemm_acc(ps, w, rhs_t, msl, tsl, is_fp8, tail=None):
            """Accumulate ps += w[:, :, msl]^T @ rhs_t[:, :, tsl] over KC
            k-chunks (DoubleRow pairs when fp8), then optional extra rows
            tail=(lhsT, rhs) closing the accumulation group."""
            if is_fp8:
                for k in range(0, KC, 2):
                    nc.tensor.matmul(ps[:], lhsT=w[:, k:k + 2, msl],
                                     rhs=rhs_t[:, k:k + 2, tsl],
                                     start=(k == 0),
                                     stop=(tail is None and k == KC - 2),
                                     perf_mode=DRM)
            else:
                for k in range(KC):
                    nc.tensor.matmul(ps[:], lhsT=w[:, k, msl],
                                     rhs=rhs_t[:, k, tsl],
                                     start=(k == 0),
                                     stop=(tail is None and k == KC - 1))
            if tail is not None:
                lt, rt = tail
                nc.tensor.matmul(ps[:], lhsT=lt, rhs=rt,
                                 start=False, stop=True)

        # ---- constants / small tiles ----
        consts = singles.tile([P, 2 * P + 64], BF16)
        nc.sync.dma_start(out=consts[:], in_=consts_d[:, :])
        cols = singles.tile([P, 24], F32)
        nc.sync.dma_start(out=cols[:], in_=cols_d[:, :])
        ext01 = singles.tile([2, C], BF16)   # rows: -c1, b1'
        nc.sync.dma_start(out=ext01[:], in_=ext_d[0:2, :])
        extb2 = singles.tile([1, C], BF16)   # b2
        nc.sync.dma_start(out=extb2[:], in_=ext_d[2:3, :])
        extbv = singles.tile([1, C], BF16)   # bev
        nc.gpsimd.dma_start(out=extbv[:], in_=ext_d[3:4, :])
        extbp = singles.tile([1, C], BF16)   # bproj
        nc.gpsimd.dma_start(out=extbp[:], in_=ext_d[4:5, :])
        onesC = singles.tile([P, 1], BF16)
        nc.vector.memset(onesC[:], 1.0 / C)
        onesR = singles.tile([1, P], BF16)
        nc.vector.memset(onesR[:], 1.0)
        # row at partition 64 for softmax normalize broadcasts; carries the
        # attnT fp8 scale so the normalize multiply quantizes for free
        onesR64 = singles.tile([Dh + 1, Dh], BF16)
        nc.vector.memset(onesR64[:], SA if FP8_PROJ else 1.0)
        onesT = singles.tile([1, T], BF16)
        nc.vector.memset(onesT[:], 1.0)
        epsc = singles.tile([P, 1], F32)
        nc.vector.memset(epsc[:], EPS)

        # LN stat rows / broadcast tiles
        m1row = singles.tile([1, T], BF16)
        r1row = singles.tile([1, T], BF16)
        mrows = singles.tile([2, T], BF16)   # p0: m2, p1: std2
        std2row = singles.tile([1, T], BF16)
        r2row = singles.tile([1, T], BF16)
        r2b = singles.tile([P, T], BF16)

        def ident():
            return consts[:, 0:P]

        def maskS():
            return consts[:, P:2 * P]

        def shiftS():
            return consts[0:Dh + 1, 2 * P:2 * P + Dh]

        with (tc.For_i(0, hwloop, 1) if hwloop else nullcontext()):
            for _it in range(loop):
                # ---------------- load x^T + first weights ----------------
                xt = big("T5", "xt")
                nc.sync.dma_start(
                    out=xt[:], in_=xt_d.ap().rearrange("p (k t) -> p k t", k=KC))
                wq = big("T2", "wq", dtype=w_dt["wq"])
                bulk_dma(out=wq[:],
                         in_=wq_d.ap().rearrange("p (k n) -> p k n", k=KC))
                wk = big("T3", "wk", dtype=w_dt["wk"])
                bulk_dma(out=wk[:],
                         in_=wk_d.ap().rearrange("p (k n) -> p k n", k=KC))
                wv = big("T4", "wv", dtype=w_dt["wv"])
                bulk_dma(out=wv[:],
                         in_=wv_d.ap().rearrange("p (k n) -> p k n", k=KC))

                # ------- LN1 stats + z, per t-half (chain hides under z/QKV) ----
                LN1 = ExitStack()
                ln1bc = LN1.enter_context(tc.tile_pool(name="ln1bc", bufs=1))
                m1b = ln1bc.tile([P, T], BF16, tag="m1b", name="m1b")
                r1b = ln1bc.tile([P, T], BF16, tag="r1b", name="r1b")
                sq = big("T0", "sq")
                for k in range(KC):
                    nc.vector.tensor_tensor(sq[:, k, :], xt[:, k, :],
                                            xt[:, k, :], ALU.mult)
                zsub = big("T9", "zsub")
                z = big("T1", "z", dtype=FP8 if FP8_QKV else BF16)
                with ExitStack() as S:
                    rtmp = S.enter_context(tc.tile_pool(name="rtmp", bufs=2))
                    for tn in range(TN):
                        tsl = slice(tn * NF, (tn + 1) * NF)
                        ps_m = stile("psm", (1, NF))
                        for k in range(KC):
                            nc.tensor.matmul(ps_m[:], lhsT=onesC[:],
                                             rhs=xt[:, k, tsl],
                                             start=(k == 0), stop=(k == KC - 1))
                        ps_s2 = stile("pss", (1, NF))
                        for k in range(KC):
                            nc.tensor.matmul(ps_s2[:], lhsT=onesC[:],
                                             rhs=sq[:, k, tsl],
                                             start=(k == 0), stop=(k == KC - 1))
                        msq = rtmp.tile([1, NF], F32, tag="msq")
                        nc.scalar.activation(out=msq[:], in_=ps_m[:],
                                             func=ACTF.Square, scale=1.0)
                        var = rtmp.tile([1, NF], F32, tag="var")
                        nc.vector.scalar_tensor_tensor(
                            out=var[:], in0=ps_s2[:], scalar=1.0, in1=msq[:],
                            op0=ALU.mult, op1=ALU.subtract)
                        stdr = rtmp.tile([1, NF], F32, tag="stdr")
                        nc.scalar.activation(out=stdr[:], in_=var[:],
                                             func=ACTF.Sqrt,
                                             bias=epsc[0:1, :], scale=1.0)
                        with nc.allow_low_precision(reason="ln rstd bf16"):
                            nc.vector.reciprocal(r1row[0:1, tsl], stdr[:])
                        nc.scalar.activation(out=m1row[0:1, tsl], in_=ps_m[:],
                                             func=ACTF.Copy, scale=1.0)
                        bc0 = stile("bc0")
                        nc.tensor.matmul(bc0[:], lhsT=onesR[0:1, :],
                                         rhs=m1row[0:1, tsl],
                                         start=True, stop=True)
                        nc.scalar.activation(out=m1b[:, tsl], in_=bc0[:],
                                             func=ACTF.Copy, scale=1.0)
                        bc1 = stile("bc1")
                        nc.tensor.matmul(bc1[:], lhsT=onesR[0:1, :],
                                         rhs=r1row[0:1, tsl],
                                         start=True, stop=True)
                        # z-quant scale folded into the rstd broadcast
                        nc.scalar.activation(out=r1b[:, tsl], in_=bc1[:],
                                             func=ACTF.Copy,
                                             scale=SZ if FP8_QKV else 1.0)
                        # z for this t-half while the other half's stats run
                        for k in range(KC):
                            nc.vector.tensor_tensor(zsub[:, k, tsl],
                                                    xt[:, k, tsl],
                                                    m1b[:, tsl], ALU.subtract)
                            nc.vector.tensor_tensor(z[:, k, tsl],
                                                    zsub[:, k, tsl],
                                                    r1b[:, tsl], ALU.mult)
                LN1.close()

                # ---------------- QKV ----------------
                if phases < 2:
                    for m in range(KC):
                        bulk_dma(out=out_d[m * P:(m + 1) * P, :],
                                 in_=zsub[:, m, :])
                    continue
                QT = big("T6", "QT")
                KT = big("T7", "KT")
                for tn in range(TN):
                    tsl = slice(tn * NF, (tn + 1) * NF)
                    for m in range(KC):
                        msl = slice(m * P, (m + 1) * P)
                        pq = stile("pq")
                        gemm_acc(pq, wq, z, msl, tsl, FP8_QKV)
                        nc.scalar.activation(out=QT[:, m, tsl], in_=pq[:],
                                             func=ACTF.Identity,
                                             bias=cols[:, m:m + 1],
                                             scale=1.0 / QKV_S)
                        pk = stile("pk")
                        gemm_acc(pk, wk, z, msl, tsl, FP8_QKV)
                        nc.scalar.activation(out=KT[:, m, tsl], in_=pk[:],
                                             func=ACTF.Identity,
                                             bias=cols[:, 8 + m:9 + m],
                                             scale=1.0 / QKV_S)

                # V natural [t(part), j, h, 65]; ones col LAST (row 64 of the
                # AV output is then the softmax denominator, rows 0:64 attn)
                V = big("TV", "V", (P, TJ, H, Dh + 1))
                nc.vector.memset(V[:, :, :, Dh:Dh + 1], 1.0)
                for j in range(TJ):
                    jsl = slice(j * P, (j + 1) * P)
                    for hn in range(TN):
                        hsl = slice(hn * NF, (hn + 1) * NF)
                        pv = stile("pv")
                        if FP8_QKV:
                            for k in range(0, KC, 2):
                                nc.tensor.matmul(pv[:], lhsT=z[:, k:k + 2, jsl],
                                                 rhs=wv[:, k:k + 2, hsl],
                                                 start=(k == 0), stop=False,
                                                 perf_mode=DRM)
                        else:
                            for k in range(KC):
                                nc.tensor.matmul(pv[:], lhsT=z[:, k, jsl],
                                                 rhs=wv[:, k, hsl],
                                                 start=(k == 0), stop=False)
                        nc.tensor.matmul(pv[:], lhsT=onesR[0:1, :],
                                         rhs=extbv[0:1, hsl],
                                         start=False, stop=True)
                        pvh = pv[:].rearrange("p (h d) -> p h d", d=Dh)
                        nc.scalar.activation(
                            out=V[:, j, hn * 8:(hn + 1) * 8, 0:Dh],
                            in_=pvh[:], func=ACTF.Copy, scale=1.0 / QKV_S)

                # prefetch late weights into freed arena space
                wp = big("T0", "wp", dtype=w_dt["wp"])
                bulk_dma(out=wp[:],
                         in_=wp_d.ap().rearrange("p (k n) -> p k n", k=KC))
                w1 = big("T1", "w1", dtype=w_dt["w1"])
                bulk_dma(out=w1[:],
                         in_=w1_d.ap().rearrange("p (k n) -> p k n", k=KC))
                w2 = big("T2", "w2", dtype=w_dt["w2"])
                bulk_dma(out=w2[:],
                         in_=w2_d.ap().rearrange("p (k n) -> p k n", k=KC))

                # ---------------- attention ----------------
                if phases < 3:
                    for m in range(KC):
                        bulk_dma(out=out_d[m * P:(m + 1) * P, :],
                                 in_=QT[:, m, :])
                    continue
                # Software-pipelined attention: per (head-pair m, t-half tn)
                # "unit", scores+exp of unit u are interleaved block-by-block
                # with the AV accumulation of unit u-1, so the PE never sits
                # at a scores->AV barrier waiting for ACT's exps and ACT is
                # fed continuously. V has the ones column LAST, so AV row 64
                # is the softmax denominator and rows 0:64 are attn for h0 —
                # the normalize multiply writes attnT[0:64] straight from
                # DVE; h1 takes one identity matmul to partitions 64:128.
                attnT = big("T8", "attnT", dtype=FP8 if FP8_PROJ else BF16)
                units = [(m, tn) for m in range(KC) for tn in range(TN)]
                with ExitStack() as S:
                    ep = S.enter_context(tc.tile_pool(name="ep", bufs=12))
                    np_ = S.enter_context(tc.tile_pool(name="np", bufs=2))

                    def score_block(m, tn, i):
                        diag = i - 4 * tn
                        d0 = P * max(diag, 0)
                        w = NF - d0
                        ssl = slice(i * P, (i + 1) * P)
                        qsl = slice(tn * NF + d0, (tn + 1) * NF)
                        ps0 = stile("ps0")
                        ps1 = stile("ps1")
                        nc.tensor.matmul(
                            ps0[:, d0:NF], lhsT=KT[0:64, m, ssl],
                            rhs=QT[0:64, m, qsl], start=True, stop=True)
                        nc.tensor.matmul(
                            ps1[:, d0:NF], lhsT=KT[64:128, m, ssl],
                            rhs=QT[64:128, m, qsl], start=True, stop=True)
                        E0 = ep.tile([P, NF], BF16, tag="E0")
                        E1 = ep.tile([P, NF], BF16, tag="E1")
                        nc.scalar.activation(out=E0[:, 0:w], in_=ps0[:, d0:NF],
                                             func=ACTF.Exp, scale=Dh ** -0.5)
                        nc.scalar.activation(out=E1[:, 0:w], in_=ps1[:, d0:NF],
                                             func=ACTF.Exp, scale=Dh ** -0.5)
                        if diag >= 0:
                            nc.vector.tensor_tensor(E0[:, 0:P], E0[:, 0:P],
                                                    maskS(), ALU.mult)
                            nc.vector.tensor_tensor(E1[:, 0:P], E1[:, 0:P],
                                                    maskS(), ALU.mult)
                        return (E0, E1, d0, w)

                    def av_block(st, i):
                        E0, E1, d0, w = st["Es"][i]
                        m, i_hi = st["m"], len(st["Es"])
                        nc.tensor.matmul(
                            st["pab0"][0:Dh + 1, d0:NF],
                            lhsT=V[:, i, 2 * m, :], rhs=E0[:, 0:w],
                            start=(i == 0), stop=(i == i_hi - 1))
                        nc.tensor.matmul(
                            st["pab1"][0:Dh + 1, d0:NF],
                            lhsT=V[:, i, 2 * m + 1, :], rhs=E1[:, 0:w],
                            start=(i == 0), stop=(i == i_hi - 1))

                    def emit_norm(st):
                        m_, tsl_ = st["m"], st["tsl"]
                        pab0, pab1 = st["pab0"], st["pab1"]
                        # one packed scratch tile: slot0/1 rows 0:64 = raw
                        # attn pair, slot0/1 row 64 = denominator recips,
                        # slot2 rows 64:128 = shifted h1
                        nt = np_.tile([P, 3, NF], BF16, tag="nt")
                        nc.vector.tensor_copy(out=nt[0:Dh, 0, :],
                                              in_=pab0[0:Dh, :])
                        nc.vector.tensor_copy(out=nt[0:Dh, 1, :],
                                              in_=pab1[0:Dh, :])
                        with nc.allow_low_precision(reason="softmax recip"):
                            nc.vector.reciprocal(nt[Dh:Dh + 1, 0, :],
                                                 pab0[Dh:Dh + 1, :])
                            nc.vector.reciprocal(nt[Dh:Dh + 1, 1, :],
                                                 pab1[Dh:Dh + 1, :])
                        rbA = ntile("NR", f"rb{m_}_{st['tn']}")
                        nc.tensor.matmul(rbA[0:Dh, :],
                                         lhsT=onesR64[Dh:Dh + 1, :],
                                         rhs=nt[Dh:Dh + 1, 0, :],
                                         start=True, stop=True)
                        nc.tensor.matmul(rbA[Dh:P, :],
                                         lhsT=onesR64[Dh:Dh + 1, :],
                                         rhs=nt[Dh:Dh + 1, 1, :],
                                         start=True, stop=True)
                        psh = ntile("NS", f"psh{m_}_{st['tn']}")
                        nc.tensor.matmul(psh[Dh:P, :],
                                         lhsT=ident()[0:Dh, 0:Dh],
                                         rhs=nt[0:Dh, 1, :],
                                         start=True, stop=True)
                        nc.scalar.activation(out=nt[Dh:P, 2, :],
                                             in_=psh[Dh:P, :],
                                             func=ACTF.Copy, scale=1.0)
                        nc.vector.tensor_tensor(attnT[0:Dh, m_, tsl_],
                                                nt[0:Dh, 0, :], rbA[0:Dh, :],
                                                ALU.mult)
                        nc.vector.tensor_tensor(attnT[Dh:P, m_, tsl_],
                                                nt[Dh:P, 2, :], rbA[Dh:P, :],
                                                ALU.mult)

                    prev = None
                    for u in range(len(units) + 1):
                        if u < len(units):
                            m, tn = units[u]
                            cur = {"m": m, "tn": tn,
                                   "tsl": slice(tn * NF, (tn + 1) * NF),
                                   "i_hi": 4 * (tn + 1), "Es": [],
                                   "pab0": atile("pa0"), "pab1": atile("pa1")}
                        else:
                            cur = None
                        n_blk = max(cur["i_hi"] if cur else 0,
                                    prev["i_hi"] if prev else 0)
                        for i in range(n_blk):
                            if cur and i < cur["i_hi"]:
                                cur["Es"].append(score_block(cur["m"],
                                                             cur["tn"], i))
                            if prev and i < prev["i_hi"]:
                                av_block(prev, i)
                        if prev:
                            emit_norm(prev)
                        prev = cur

                # ---------------- proj + residual -> y^T ----------------
                if phases < 4:
                    for m in range(KC):
                        bulk_dma(out=out_d[m * P:(m + 1) * P, :],
                                 in_=QT[:, m, :])
                    continue
                # proj per t-half with the LN2 stat chain for each half
                # emitted right after it (chain latency hides under the next
                # half's GEMMs / fc1's DR matmuls)
                yT = big("T3", "yT")
                sq2 = big("T6", "sq2")
                with ExitStack() as S:
                    rtmp = S.enter_context(tc.tile_pool(name="rtmp2", bufs=2))
                    for tn in range(TN):
                        tsl = slice(tn * NF, (tn + 1) * NF)
                        for m in range(KC):
                            msl = slice(m * P, (m + 1) * P)
                            pp = stile("pp")
                            gemm_acc(pp, wp, attnT, msl, tsl, FP8_PROJ,
                                     tail=(extbp[0:1, msl], onesT[0:1, tsl]))
                            nc.vector.scalar_tensor_tensor(
                                out=yT[:, m, tsl], in0=pp[:],
                                scalar=1.0 / PROJ_S, in1=xt[:, m, tsl],
                                op0=ALU.mult, op1=ALU.add)
                        if phases < 5:
                            continue
                        # ---- LN2 stats for this t-half ----
                        for k in range(KC):
                            nc.vector.tensor_tensor(sq2[:, k, tsl],
                                                    yT[:, k, tsl],
                                                    yT[:, k, tsl], ALU.mult)
                        ps_m = stile("psm2", (1, NF))
                        for k in range(KC):
                            nc.tensor.matmul(ps_m[:], lhsT=onesC[:],
                                             rhs=yT[:, k, tsl],
                                             start=(k == 0), stop=(k == KC - 1))
                        ps_s2 = stile("pss2", (1, NF))
                        for k in range(KC):
                            nc.tensor.matmul(ps_s2[:], lhsT=onesC[:],
                                             rhs=sq2[:, k, tsl],
                                             start=(k == 0), stop=(k == KC - 1))
                        msq = rtmp.tile([1, NF], F32, tag="msq")
                        nc.scalar.activation(out=msq[:], in_=ps_m[:],
                                             func=ACTF.Square, scale=1.0)
                        var = rtmp.tile([1, NF], F32, tag="var")
                        nc.vector.scalar_tensor_tensor(
                            out=var[:], in0=ps_s2[:], scalar=1.0, in1=msq[:],
                            op0=ALU.mult, op1=ALU.subtract)
                        # std2 (bf16); copy to mrows p1 via 1-ch broadcast shift
                        nc.scalar.activation(out=std2row[0:1, tsl], in_=var[:],
                                             func=ACTF.Sqrt,
                                             bias=epsc[0:1, :], scale=1.0)
                        nc.gpsimd.dma_start(out=mrows[1:2, tsl],
                                          in_=std2row[0:1, tsl])
                        with nc.allow_low_precision(reason="ln2 rstd bf16"):
                            nc.vector.reciprocal(r2row[0:1, tsl],
                                                 std2row[0:1, tsl])
                        nc.scalar.activation(out=mrows[0:1, tsl], in_=ps_m[:],
                                             func=ACTF.Copy, scale=1.0)
                        bc2 = stile("bc2")
                        nc.tensor.matmul(bc2[:], lhsT=onesR[0:1, :],
                                         rhs=r2row[0:1, tsl],
                                         start=True, stop=True)
                        # fc2 descale folded into the rstd broadcast
                        nc.scalar.activation(out=r2b[:, tsl], in_=bc2[:],
                                             func=ACTF.Copy,
                                             scale=1.0 / MLP2_S)

                if phases < 5:
                    for m in range(KC):
                        bulk_dma(out=out_d[m * P:(m + 1) * P, :],
                                 in_=yT[:, m, :])
                    continue

                # ---------------- MLP fc1 ----------------
                if phases < 6:
                    for m in range(KC):
                        bulk_dma(out=out_d[m * P:(m + 1) * P, :],
                                 in_=yT[:, m, :])
                    continue
                if FP8_MLP1:
                    yq = big("T9", "yq", dtype=FP8)
                    for k in range(KC):
                        nc.vector.tensor_scalar_mul(yq[:, k, :], yT[:, k, :],
                                                    SY)
                else:
                    yq = yT
                hT = big("T4", "hT", dtype=FP8 if FP8_MLP2 else BF16)
                relu_s = (SH if FP8_MLP2 else 1.0) / MLP1_S
                for tn in range(TN):
                    tsl = slice(tn * NF, (tn + 1) * NF)
                    for m in range(KC):
                        msl = slice(m * P, (m + 1) * P)
                        ph = stile("ph")
                        gemm_acc(ph, w1, yq, msl, tsl, FP8_MLP1,
                                 tail=(ext01[0:2, msl], mrows[0:2, tsl]))
                        nc.scalar.activation(out=hT[:, m, tsl], in_=ph[:],
                                             func=ACTF.Relu, scale=relu_s)

                # ---------------- MLP fc2 + residual -> out^T ----------------
                if phases < 7:
                    for m in range(KC):
                        bulk_dma(out=out_d[m * P:(m + 1) * P, :],
                                 in_=hT[:, m, :])
                    continue
                osb = big("T5", "osb")
                with ExitStack() as S:
                    otp = S.enter_context(tc.tile_pool(name="otp", bufs=3))
                    for m in range(KC):
                        msl = slice(m * P, (m + 1) * P)
                        for tn in range(TN):
                            tsl = slice(tn * NF, (tn + 1) * NF)
                            po = stile("po")
                            gemm_acc(po, w2, hT, msl, tsl, FP8_MLP2,
                                     tail=(extb2[0:1, msl], std2row[0:1, tsl]))
                            tmp = otp.tile([P, NF], BF16, tag="tmp")
                            nc.vector.tensor_tensor(tmp[:], po[:],
                                                    r2b[:, tsl], ALU.mult)
                            nc.vector.tensor_tensor(osb[:, m, tsl], tmp[:],
                                                    yT[:, m, tsl], ALU.add)
                        bulk_dma(out=out_d[m * P:(m + 1) * P, :],
                                 in_=osb[:, m, :])

    nc.compile()
    return nc


def _prep_inputs(inputs):
    """Host-side prep: dtype casts, transposes, LN gain/bias folds, fp8."""
    f = np.float32
    bf = ml_dtypes.bfloat16
    f8 = ml_dtypes.float8_e4m3
    x = np.asarray(inputs["x"], dtype=f)                       # [B, T, C]
    g1 = np.asarray(inputs["g1"], dtype=f)
    be1 = np.asarray(inputs["beta1"], dtype=f)
    g2 = np.asarray(inputs["g2"], dtype=f)
    be2 = np.asarray(inputs["beta2"], dtype=f)
    Wq = np.asarray(inputs["Wq"], dtype=f).transpose(1, 0, 2).reshape(C, C)
    Wk = np.asarray(inputs["Wk"], dtype=f).transpose(1, 0, 2).reshape(C, C)
    Wv = np.asarray(inputs["Wv"], dtype=f).transpose(1, 0, 2).reshape(C, C)
    Wp = np.asarray(inputs["Wproj"], dtype=f)
    W1 = np.asarray(inputs["W1"], dtype=f)
    W2 = np.asarray(inputs["W2"], dtype=f)
    b1 = np.asarray(inputs["b1"], dtype=f)
    b2 = np.asarray(inputs["b2"], dtype=f)
    bp = np.asarray(inputs["bproj"], dtype=f)

    def shuf(a, dt):
        # [C, X] -> [P, KC*X]: DRAM row p holds chunks k at [k*X:(k+1)*X]
        X = a.shape[1]
        return np.ascontiguousarray(
            a.reshape(KC, P, X).transpose(1, 0, 2).reshape(P, KC * X)
        ).astype(dt)

    def wprep(a, is_fp8):
        if is_fp8:
            return shuf(np.clip(a * SW, -240, 240), f8)
        return shuf(a, bf)

    w1g = g2[:, None] * W1
    w1_dev = wprep(w1g, FP8_MLP1)
    # -c1 from the device-visible (quantized) w1 for exact mean cancellation
    if FP8_MLP1:
        w1_eff = w1_dev.astype(f).reshape(P, KC, C).transpose(1, 0, 2) \
            .reshape(C, C) / SW
    else:
        w1_eff = w1_dev.astype(f).reshape(P, KC, C).transpose(1, 0, 2) \
            .reshape(C, C)

    cols = np.zeros((P, 24), f)
    cols[:, 0:8] = (be1 @ Wq).reshape(KC, P).T
    cols[:, 8:16] = (be1 @ Wk).reshape(KC, P).T
    ext = np.zeros((5, C), f)
    ext[0] = -np.sum(w1_eff, axis=0) * MLP1_S
    ext[1] = (b1 + be2 @ W1) * MLP1_S
    ext[2] = b2 * MLP2_S
    ext[3] = (be1 @ Wv) * QKV_S
    ext[4] = bp * PROJ_S
    consts = np.zeros((P, 2 * P + 64), f)
    consts[:, 0:P] = np.eye(P, dtype=f)
    consts[:, P:2 * P] = np.where(
        np.arange(P)[:, None] <= np.arange(P)[None, :], 1.0, 0.0)
    # shiftS[i, j] = 1 iff i == j+1: out[j] = in[j+1] (drop sum row 0)
    consts[0:Dh + 1, 2 * P:2 * P + Dh] = np.eye(Dh + 1, Dh, k=-1, dtype=f)

    common = {
        "wq": wprep(g1[:, None] * Wq, FP8_QKV),
        "wk": wprep(g1[:, None] * Wk, FP8_QKV),
        "wv": wprep(g1[:, None] * Wv, FP8_QKV),
        "wp": wprep(Wp, FP8_PROJ),
        "w1": w1_dev,
        "w2": wprep(W2, FP8_MLP2),
        "cols": cols,
        "ext": ext.astype(bf),
        "consts": consts.astype(bf),
    }
    return [{"xt": shuf(np.ascontiguousarray(x[b].T), bf), **common}
            for b in range(N_CORES)]


def kernel(**inputs) -> np.ndarray:
    if "nc" not in _CACHE:
        _CACHE["nc"] = build_nc()
    nc = _CACHE["nc"]
    in_maps = _prep_inputs(inputs)
    res = run_bass_kernel_spmd(nc, in_maps, list(range(N_CORES)))
    out = np.stack(
        [np.asarray(res.results[b]["out"]).astype(np.float32).T
         for b in range(N_CORES)], axis=0)
    return np.ascontiguousarray(out)


if __name__ == "__main__":
    rng = np.random.default_rng(0)
    demo = {
        "x": rng.standard_normal((B, T, C), dtype=np.float32),
        "Wq": rng.standard_normal((H, C, Dh), dtype=np.float32) * 0.02,
        "Wk": rng.standard_normal((H, C, Dh), dtype=np.float32) * 0.02,
        "Wv": rng.standard_normal((H, C, Dh), dtype=np.float32) * 0.02,
        "Wproj": rng.standard_normal((C, C), dtype=np.float32) * 0.02,
        "bproj": np.zeros(C, np.float32),
        "W1": rng.standard_normal((C, C), dtype=np.float32) * 0.02,
        "b1": np.zeros(C, np.float32),
        "W2": rng.standard_normal((C, C), dtype=np.float32) * 0.02,
        "b2": np.zeros(C, np.float32),
        "g1": np.ones(C, np.float32),
        "beta1": np.zeros(C, np.float32),
        "g2": np.ones(C, np.float32),
        "beta2": np.zeros(C, np.float32),
    }
    y = kernel(**demo)
    print("out", y.shape, y.dtype, float(np.abs(y).max()))
